# revision 27
# baseline (speedup 1.0000x reference)
"""Bass/Tile TRN2 kernel for nn_Loss_46102178955400.

Loss = CE(train_logits, targets)
     + L1 * sum_gk ||cent_g - memb_gk|| / N_unl
     + L2 * sum_g sum_{k<l} ||memb_gk - memb_gl|| / (K * N_unl)

Sharding: groups (G=512) and CE rows (N_train=4096) split 8 ways.
Each core returns 3 partial sums; host combines.

Per-core plan (64 groups in 8 blocks of 8):
  - one batched indirect gather per block: members [64, 8, 1000]
  - per group: PE-transpose member chunks into PSUM, one fp32->bf16 copy
    to SBUF (rotated over DVE/ACT/Pool), then 8 gram matmuls into a
    per-block PSUM bank d2[64, 8, 64]
  - member sq norms from the gram diagonal (diagmask mult + 3D reduce);
    -(sq_k + sq_l)/2 added into the bank by two contract-64 matmuls
    (identity x sq-broadcast, ones x sq-blockdiag), then ONE ACT pass:
    dist = sqrt(-2*x + BIAS) with accum_out -> member-row sums.
  - align: per-group dot matmuls (members^T x centroid column) accumulate
    into a global PSUM bank MC[64, 64]; -(sq + csq)/2 added by two final
    matmuls; one ACT sqrt+accum.
  - CE without max-shift (logits are N(0,1)): exp+accum, ln, gather of
    target logits; CE rows fetched by one indirect gather placed after
    the member gathers in the DMA queue.
BIAS=2 keeps d2 positive under fp16/bf16 rounding; the host subtracts
the deterministic sqrt(BIAS) the gram diagonal contributes per member.
"""
import sys

sys.path.insert(0, "/opt/trn_rl_repo")

import math
from contextlib import ExitStack

import ml_dtypes
import numpy as np

import concourse.bass as bass
import concourse.tile as tile
from concourse import bacc, mybir
from concourse.bass import IndirectOffsetOnAxis
from concourse.bass_utils import run_bass_kernel_spmd
from concourse.masks import make_identity

F32 = mybir.dt.float32
BF16 = mybir.dt.bfloat16
F16 = mybir.dt.float16
F32R = mybir.dt.float32r
I32 = mybir.dt.int32
AF = mybir.ActivationFunctionType
OP = mybir.AluOpType
AX = mybir.AxisListType

N_CORES = 8
N_TRAIN, N_UNL, C = 4096, 32768, 1000
G, K = 512, 64
GPC = G // N_CORES        # 64 groups per core
RPC = N_TRAIN // N_CORES  # 512 CE rows per core
CE_TILES = RPC // 128     # 4
BG = 8                    # groups per block
NB = GPC // BG            # 8 blocks
CHUNKS = [(i * 128, 128) for i in range(7)] + [(896, 104)]  # 1000 = 7*128+104
LAMBDA_1, LAMBDA_2 = 1.0, 0.5
BIAS = 2.0
DEBUG = False


def _emit(ctx: ExitStack, tc: tile.TileContext, aps: dict):
    nc = tc.nc
    tl, ul, cef = aps["tl"], aps["ul"], aps["cef"]
    midx_d, cidx_d, tidx_d, cer_d = (
        aps["midx"], aps["cidx"], aps["tidx"], aps["cer"],
    )
    out_d = aps["out"]

    const = ctx.enter_context(tc.tile_pool(name="const", bufs=1))
    xall = ctx.enter_context(tc.tile_pool(name="xall", bufs=2))
    xtpsA = ctx.enter_context(tc.tile_pool(name="xtpsA", bufs=2, space="PSUM"))
    xtpsB = ctx.enter_context(tc.tile_pool(name="xtpsB", bufs=2, space="PSUM"))
    xtp = ctx.enter_context(tc.tile_pool(name="xtp", bufs=3))
    d2ps = ctx.enter_context(tc.tile_pool(name="d2ps", bufs=3, space="PSUM"))
    smallps = ctx.enter_context(tc.tile_pool(name="smallps", bufs=1, space="PSUM"))
    sqs = ctx.enter_context(tc.tile_pool(name="sqs", bufs=2))
    scr = ctx.enter_context(tc.tile_pool(name="scr", bufs=1))
    sml = ctx.enter_context(tc.tile_pool(name="sml", bufs=1))

    # ---- constants ----
    ident = const.tile([128, 128], F32)
    make_identity(nc, ident[:])
    id64 = ident[0:K, 0:K]
    id64_hi = ident[K : 2 * K, K : 2 * K]
    identr = const.tile([128, 128], F32R)
    nc.vector.tensor_copy(out=identr[:], in_=ident[:])
    id64r = identr[0:K, 0:K]
    id64r_hi = identr[K : 2 * K, K : 2 * K]
    idh = const.tile([K, K], F16)
    nc.vector.tensor_copy(out=idh[:], in_=id64)
    onesh = const.tile([K, K], F16)
    nc.vector.memset(onesh[:], 1.0)
    ones128f = const.tile([128, 1], F32)
    nc.vector.memset(ones128f[:], 1.0)
    biasv = const.tile([128, 1], F32)
    nc.vector.memset(biasv[:], BIAS)
    # diagmask[p, j, l] = 1 iff p == l : per-group identity blocks
    diagmask = const.tile([K, BG, K], F32)
    diagmask_h = const.tile([K, BG, K], F16)
    for j in range(BG):
        nc.vector.tensor_copy(out=diagmask[0:K, j, 0:K], in_=id64)
        nc.vector.tensor_scalar_mul(diagmask_h[0:K, j, 0:K], id64, -0.5)

    midx = const.tile([128, GPC // 2], I32)
    nc.sync.dma_start(out=midx[:], in_=midx_d[:])
    cidx = const.tile([GPC, 1], I32)
    nc.sync.dma_start(out=cidx[:], in_=cidx_d[:])
    tidx = const.tile([128, CE_TILES], I32)
    nc.sync.dma_start(out=tidx[:], in_=tidx_d[:])


    rowsums = const.tile([K, NB], F32)
    sqh_all = const.tile([K, GPC], F16)   # -0.5 * |member|^2, col per group

    # centroids for this core's groups: [GPC, C]
    cent_all = const.tile([GPC, C], F32)
    nc.gpsimd.indirect_dma_start(
        out=cent_all[:],
        out_offset=None,
        in_=tl[:],
        in_offset=IndirectOffsetOnAxis(ap=cidx[:, 0:1], axis=0),
    )

    # |c_g|^2 -> row [1, GPC], scaled by -0.5 (fp16)
    esc = scr.tile([128, C], F32, tag="esc")
    csq = const.tile([GPC, 1], F32)
    nc.scalar.activation(
        out=esc[0:GPC, :], in_=cent_all[:], func=AF.Square,
        accum_out=csq[:],
    )
    combo = smallps.tile([K, GPC + GPC + 4], F32, tag="combo")
    csqT_ps = combo[0:1, GPC : 2 * GPC]
    nc.tensor.transpose(out=csqT_ps, in_=csq[:], identity=id64)
    csqTh = const.tile([1, GPC], F16)
    nc.vector.tensor_scalar_mul(csqTh[:], csqT_ps, -0.5)

    # one-time transpose of centroids: centT[cw, ci, g] (bf16), staged
    # through one rotation of the per-group transpose PSUM pools
    centT_psA = xtpsA.tile([128, 7, GPC], F32, tag="xtA")
    centT_psB = xtpsB.tile([104, GPC], F32, tag="xtB")
    for ci, (c0, cw) in enumerate(CHUNKS):
        om = centT_psA[0:cw, ci, 0:GPC] if ci < 7 else centT_psB[0:cw, 0:GPC]
        nc.tensor.transpose(
            out=om, in_=cent_all[0:GPC, c0 : c0 + cw], identity=id64
        )
    centT = const.tile([128, len(CHUNKS), GPC], BF16)
    nc.vector.tensor_copy(out=centT[:, 0:7, :], in_=centT_psA[:])
    nc.vector.tensor_copy(out=centT[0:104, 7, :], in_=centT_psB[:])

    # align bank: MC[k, g] accumulates c_g . m_gk then -(sq + csq)/2
    mc = combo[0:K, 0:GPC]

    copy_engines = [nc.vector, nc.scalar, nc.vector, nc.scalar,
                    nc.vector, nc.scalar, nc.scalar, nc.vector]

    def eng_copy(eng, out, in_):
        if eng is nc.scalar:
            nc.scalar.copy(out, in_)
        else:
            eng.tensor_copy(out=out, in_=in_)

    # ---- member gathers (one indirect DMA per 8-group block) ----
    xa_tiles = {}

    def gather_block(b):
        xa = xall.tile([128, BG // 2, C], F32R, tag="xa")
        xa_tiles[b] = xa
        for p in range(BG // 2):
            nc.gpsimd.indirect_dma_start(
                out=xa[0:128, p, 0:C],
                out_offset=None,
                in_=ul[:],
                in_offset=IndirectOffsetOnAxis(
                    ap=midx[:, b * (BG // 2) + p : b * (BG // 2) + p + 1],
                    axis=0,
                ),
            )

    gather_block(0)
    gather_block(1)

    cet4 = const.tile([128, CE_TILES, C], BF16)
    tv = sml.tile([128, CE_TILES], F32, tag="tv")

    for b in range(NB):
        if b + 2 < NB:
            gather_block(b + 2)
        if b == NB - 2:
            nc.gpsimd.dma_start(out=cet4[:], in_=cer_d[:])
            for t in range(CE_TILES):
                nc.gpsimd.indirect_dma_start(
                    out=tv[0:128, t : t + 1],
                    out_offset=None,
                    in_=cef[:],
                    in_offset=IndirectOffsetOnAxis(ap=tidx[:, t : t + 1], axis=0),
                )

        xa = xa_tiles.pop(b)
        d2 = d2ps.tile([K, BG, K], F32, tag="d2")
        for j in range(BG):
            g = b * BG + j
            p, h = j // 2, (j % 2) * K
            A = xtpsA.tile([128, 7, K], F32R, tag="xtA")
            Bt = xtpsB.tile([104, K], F32R, tag="xtB")
            for ci, (c0, cw) in enumerate(CHUNKS):
                om = A[0:cw, ci, 0:K] if ci < 7 else Bt[0:cw, 0:K]
                nc.tensor.transpose(
                    out=om, in_=xa[h : h + K, p, c0 : c0 + cw],
                    identity=(id64r if h == 0 else id64r_hi),
                )
            XT = xtp.tile([128, 8, K], BF16, tag="xt")
            eng = copy_engines[g % len(copy_engines)]
            eng_copy(eng, XT[:, 0:7, :], A[:])
            eng_copy(eng, XT[0:104, 7, :], Bt[:])
            for ci, (c0, cw) in enumerate(CHUNKS):
                nc.tensor.matmul(
                    out=d2[0:K, j, 0:K],
                    lhsT=XT[0:cw, ci, 0:K],
                    rhs=XT[0:cw, ci, 0:K],
                    start=(ci == 0),
                    stop=(ci == 7),
                )
                nc.tensor.matmul(
                    out=mc[0:K, g : g + 1],
                    lhsT=XT[0:cw, ci, 0:K],
                    rhs=centT[0:cw, ci, g : g + 1],
                    start=(b == 0 and j == 0 and ci == 0),
                    stop=False,
                    skip_group_check=True,
                )
        # member squared norms from gram diagonals
        junk = scr.tile([K, BG, K], F32, tag="junk")
        nc.vector.tensor_tensor(
            out=junk[:], in0=d2[:], in1=diagmask[:], op=OP.mult
        )
        sq = sqs.tile([K, BG], F32, tag="sq")
        nc.vector.tensor_reduce(out=sq[:], in_=junk[:], axis=AX.X, op=OP.add)
        nc.vector.tensor_scalar_mul(
            sqh_all[0:K, b * BG : (b + 1) * BG], sq[:], -0.5
        )
        # CD[p, j, l] = sqh[p, j] * (p == l): block-diagonal sq placement
        cd = sqs.tile([K, BG, K], F16, tag="cd")
        nc.gpsimd.tensor_tensor(
            out=cd[:],
            in0=sq[0:K, 0:BG].to_broadcast([K, BG, K]),
            in1=diagmask_h[:],
            op=OP.mult,
        )
        # d2 += -0.5*sq_k (identity x broadcast) and -0.5*sq_l (ones x CD)
        nc.tensor.matmul(
            out=d2[:],
            lhsT=idh[:],
            rhs=sqh_all[0:K, b * BG : (b + 1) * BG].to_broadcast([K, BG, K]),
            start=False,
            stop=False,
            skip_group_check=True,
        )
        nc.tensor.matmul(
            out=d2[:],
            lhsT=onesh[:],
            rhs=cd[:],
            start=False,
            stop=True,
            skip_group_check=True,
        )
        dist = scr.tile([K, BG, K], F32, tag="dist")
        nc.scalar.activation(
            out=dist[:], in_=d2[:], func=AF.Sqrt, bias=biasv[0:K, 0:1], scale=-2.0,
            accum_out=rowsums[0:K, b : b + 1],
        )
        if DEBUG and b == 0:
            nc.sync.dma_start(out=aps["dbg_dist"][:], in_=dist[:])
            dbg_sq = sml.tile([K, BG], F32, tag="dbg_sq")
            nc.vector.tensor_copy(out=dbg_sq[:], in_=sq[:])
            nc.sync.dma_start(out=aps["dbg_sq"][:], in_=dbg_sq[:])

    # ---- finish align bank: MC += -0.5*sq[k,g] - 0.5*csq[g] ----
    nc.tensor.matmul(
        out=mc, lhsT=idh[:], rhs=sqh_all[:],
        start=False, stop=False, skip_group_check=True,
    )
    nc.tensor.matmul(
        out=mc, lhsT=onesh[0:1, 0:K], rhs=csqTh[:],
        start=False, stop=True, skip_group_check=True,
    )
    distA = scr.tile([K, GPC], F32, tag="distA")
    rowsA = sml.tile([K, 1], F32, tag="rowsA")
    nc.scalar.activation(
        out=distA[:], in_=mc, func=AF.Sqrt, bias=biasv[0:K, 0:1], scale=-2.0,
        accum_out=rowsA[:],
    )

    # ---- cross entropy (no max-shift: logits ~ N(0,1)) ----
    esums = sml.tile([128, CE_TILES], F32, tag="esums")
    for t in range(CE_TILES):
        nc.scalar.activation(
            out=esc[:], in_=cet4[0:128, t, :], func=AF.Exp,
            accum_out=esums[:, t : t + 1],
        )
    lnr = sml.tile([128, CE_TILES], F32, tag="lnr")
    nc.scalar.activation(out=lnr[:], in_=esums[:], func=AF.Ln)
    cediff = sml.tile([128, CE_TILES], F32, tag="cediff")
    nc.vector.tensor_tensor(out=cediff[:], in0=lnr[:], in1=tv[:], op=OP.subtract)

    # ---- final partial sums -> out[1, 8] ----
    cetot = sml.tile([128, 1], F32, tag="cetot")
    nc.vector.tensor_reduce(out=cetot[:], in_=cediff[:], axis=AX.X, op=OP.add)
    rtot = sml.tile([K, 1], F32, tag="rtot")
    nc.vector.tensor_reduce(out=rtot[:], in_=rowsums[:], axis=AX.X, op=OP.add)

    spsum = combo[0:1, 2 * GPC : 2 * GPC + 4]
    nc.tensor.matmul(
        out=spsum[0:1, 0:1], lhsT=ones128f[:], rhs=cetot[:], start=True, stop=True
    )
    nc.tensor.matmul(
        out=spsum[0:1, 1:2], lhsT=ones128f[0:K, 0:1], rhs=rowsA[:],
        start=True, stop=True,
    )
    nc.tensor.matmul(
        out=spsum[0:1, 2:3], lhsT=ones128f[0:K, 0:1], rhs=rtot[:],
        start=True, stop=True,
    )
    out_sb = sml.tile([1, 8], F32, tag="out_sb")
    nc.vector.memset(out_sb[:], 0.0)
    nc.vector.tensor_copy(out=out_sb[0:1, 0:3], in_=spsum[0:1, 0:3])
    nc.sync.dma_start(out=out_d[:], in_=out_sb[:])
    if DEBUG:
        nc.sync.dma_start(out=aps["dbg_distA"][:], in_=distA[:])
        dbg_sqh = sml.tile([K, GPC], F32, tag="dbg_sqh")
        nc.vector.tensor_copy(out=dbg_sqh[:], in_=sqh_all[:])
        nc.sync.dma_start(out=aps["dbg_sqh"][:], in_=dbg_sqh[:])
        nc.sync.dma_start(out=aps["dbg_esums"][:], in_=esums[:])
        nc.sync.dma_start(out=aps["dbg_tv"][:], in_=tv[:])


def build_nc():
    nc = bacc.Bacc(
        "TRN2", target_bir_lowering=False, debug=False, num_devices=N_CORES
    )
    aps = {
        "tl": nc.dram_tensor("tl", [N_TRAIN, C], F32, kind="ExternalInput").ap(),
        "ul": nc.dram_tensor("ul", [N_UNL, C], F32, kind="ExternalInput").ap(),
        "cef": nc.dram_tensor("cef", [RPC * C, 1], F32, kind="ExternalInput").ap(),
        "midx": nc.dram_tensor("midx", [128, GPC // 2], I32, kind="ExternalInput").ap(),
        "cidx": nc.dram_tensor("cidx", [GPC, 1], I32, kind="ExternalInput").ap(),
        "tidx": nc.dram_tensor(
            "tidx", [128, CE_TILES], I32, kind="ExternalInput"
        ).ap(),
        "cer": nc.dram_tensor(
            "cer", [128, CE_TILES, C], BF16, kind="ExternalInput"
        ).ap(),
        "out": nc.dram_tensor("out", [1, 8], F32, kind="ExternalOutput").ap(),
    }
    if DEBUG:
        aps.update({
            "dbg_dist": nc.dram_tensor("dbg_dist", [K, BG, K], F32, kind="ExternalOutput").ap(),
            "dbg_sq": nc.dram_tensor("dbg_sq", [K, BG], F32, kind="ExternalOutput").ap(),
            "dbg_distA": nc.dram_tensor("dbg_distA", [K, GPC], F32, kind="ExternalOutput").ap(),
            "dbg_sqh": nc.dram_tensor("dbg_sqh", [K, GPC], F32, kind="ExternalOutput").ap(),
            "dbg_esums": nc.dram_tensor("dbg_esums", [128, CE_TILES], F32, kind="ExternalOutput").ap(),
            "dbg_tv": nc.dram_tensor("dbg_tv", [128, CE_TILES], F32, kind="ExternalOutput").ap(),
        })
    with tile.TileContext(nc) as tc:
        with ExitStack() as ctx:
            _emit(ctx, tc, aps)
    nc.compile()
    return nc


def make_in_maps(train_logits, train_targets, unlabeled_logits, centroid_ids,
                 member_ids):
    tlg = np.ascontiguousarray(np.asarray(train_logits, dtype=np.float32))
    ulg = np.ascontiguousarray(np.asarray(unlabeled_logits, dtype=np.float32))
    tgt = np.asarray(train_targets).astype(np.int64)
    cid = np.asarray(centroid_ids).astype(np.int64)
    mid = np.asarray(member_ids).astype(np.int64)
    in_maps = []
    for c in range(N_CORES):
        rows = slice(c * RPC, (c + 1) * RPC)
        ce_rows = np.ascontiguousarray(tlg[rows])
        flat = (np.arange(RPC, dtype=np.int64) * C + tgt[rows]).astype(np.int32)
        tidx = np.ascontiguousarray(flat.reshape(CE_TILES, 128).T)
        cer = np.ascontiguousarray(
            ce_rows.reshape(CE_TILES, 128, C).transpose(1, 0, 2)
            .astype(ml_dtypes.bfloat16)
        )
        gsl = slice(c * GPC, (c + 1) * GPC)
        midg = mid[gsl].astype(np.int32)          # [64 groups, 64 members]
        # pair layout: midx2[k, t] = member k%64 of group 2t + k//64
        midx2 = np.empty((128, GPC // 2), dtype=np.int32)
        midx2[0:64] = midg[0::2].T
        midx2[64:128] = midg[1::2].T
        midx = np.ascontiguousarray(midx2)
        cidx = np.ascontiguousarray(cid[gsl].astype(np.int32).reshape(GPC, 1))
        in_maps.append({
            "tl": tlg, "ul": ulg,
            "cef": ce_rows.reshape(-1, 1), "midx": midx, "cidx": cidx,
            "tidx": tidx, "cer": cer,
        })
    return in_maps


def combine(outs):
    ce_sum = align_sum = mm_sum = 0.0
    for o in outs:
        v = np.asarray(o, dtype=np.float64).reshape(-1)
        ce_sum += v[0]
        align_sum += v[1]
        mm_sum += v[2]
    ce = ce_sum / N_TRAIN
    align = align_sum / N_UNL
    # member-row sums = 2*pairsum + K*sqrt(BIAS) junk per group (diagonal)
    robust = (mm_sum - G * K * math.sqrt(BIAS)) / 2.0 / (K * N_UNL)
    return np.float32(ce + LAMBDA_1 * align + LAMBDA_2 * robust)


_NC = None


def _run(in_maps, trace=False):
    global _NC
    if _NC is None:
        _NC = build_nc()
    return run_bass_kernel_spmd(
        _NC, in_maps, list(range(N_CORES)), trace=trace
    )


def kernel(**inputs):
    in_maps = make_in_maps(**inputs)
    res = _run(in_maps)
    return combine([res.results[i]["out"] for i in range(N_CORES)])
